# revision 38
# baseline (speedup 1.0000x reference)
"""Trainium2 Bass kernel: 4096x4096 fp32 'valid' cross-correlation with a 15x15
kernel, plus scalar bias.

Strategy
--------
- Shard the output 2x4 across 8 NeuronCores: 2 W-stripes of 2048 cols x 4
  H-bands of 1026 rows (4x1026 >= 4082; tails trimmed on the host). Each
  core's input is its stripe/band plus a 14-pixel halo on each axis, gathered
  on the host from a zero-padded copy -- no device-to-device communication.
  The wide stripes make every DMA ~0.5 MB (4 KB/partition), near line rate;
  the old 512-col stripes moved 134 KB per DMA at ~30% efficiency.
- Per core, 9 h-chunks of 114 output rows: a [K<=128, 114] banded-Toeplitz
  stationary (T_dj[k, m] = weight[k-m, dj]) contracts 128 input rows against
  114 output rows; the W-shift for dj is a free-dim offset in the moving
  operand (image rows in SBUF partitions, W along free). 15 dj passes x 4
  512-col blocks per chunk accumulate in PSUM.
- bf16 operands (1 cycle/row on the PE vs 4 for fp32; fp32 PSUM accumulation
  keeps rel err ~3e-3 << 2e-2), bf16 output (halves store traffic vs fp32;
  upcast on host).
- Input DMAs issue on the SP HWDGE ring (nc.sync), output DMAs on the ACT
  ring (nc.scalar) so loads and stores overlap instead of sharing one FIFO.
"""

import numpy as np

H, W = 4096, 4096
KH, KW = 15, 15
HO, WO = H - KH + 1, W - KW + 1  # 4082, 4082
NCORES = 8
WSH, HSH = 2, 4          # core grid: 2 W-stripes x 4 H-bands
C = 2048                 # output cols per stripe
CIN = C + KW - 1         # input cols per stripe (with halo) = 2062
MCH = 114                # output rows per h-chunk (114 + 14 = 128 = K)
NCHUNK = 9               # chunks per band
B = NCHUNK * MCH         # output rows per band = 1026
BIN = B + KH - 1         # input rows per band = 1040
NBLK = C // 512          # 512-col psum blocks per chunk
XR_PAD = HSH * B + KH - 1   # padded input rows = 4118
XC_PAD = WSH * C + KW - 1   # padded input cols = 4110

_CACHE = {}


def _bf16():
    import ml_dtypes
    return ml_dtypes.bfloat16


def _build_nc(reps: int = 1, n_dj: int = KW, hw_loop: bool = False,
              parts: tuple = ("in", "mm", "drain", "out")):
    import concourse.bacc as bacc
    import concourse.mybir as mybir
    from concourse.tile import TileContext

    parts = set(parts)
    f32 = mybir.dt.float32
    bf16 = mybir.dt.bfloat16

    nc = bacc.Bacc("TRN2", debug=False, num_devices=NCORES)
    xs_d = nc.dram_tensor("xs", [BIN, CIN], bf16, kind="ExternalInput")
    wT_d = nc.dram_tensor("wT", [128, KW, 128], bf16, kind="ExternalInput")
    bias_d = nc.dram_tensor("bias", [1, 1], f32, kind="ExternalInput")
    ys_d = nc.dram_tensor("ys", [B, C], bf16, kind="ExternalOutput")

    with TileContext(nc) as tc:
        with (
            tc.tile_pool(name="xp", bufs=2) as xp,
            tc.tile_pool(name="wp", bufs=1) as wp,
            tc.tile_pool(name="op", bufs=3) as op,
            tc.tile_pool(name="pp", bufs=6, space="PSUM") as pp,
        ):
            # Weights (Toeplitz stack, M padded to 128 cols for FWL) + bias
            w_t = wp.tile([128, KW, 128], bf16)
            nc.sync.dma_start(w_t[:, :, :], wT_d[:, :, :])
            bias_t = wp.tile([1, 1], f32)
            nc.sync.dma_start(bias_t[:, :], bias_d[:, :])
            bias_bc = wp.tile([128, 1], f32)
            nc.gpsimd.partition_broadcast(bias_bc[:, :], bias_t[:, :])

            # Static stand-ins for isolated-stage probe builds
            x_s = o_s = None
            if "mm" in parts and "in" not in parts:
                x_s = wp.tile([128, CIN], bf16)
                nc.sync.dma_start(x_s[:, :], xs_d[0:128, :])
            if "out" in parts and "drain" not in parts:
                o_s = wp.tile([MCH, C], bf16)
                nc.vector.memset(o_s[:, :], 0.0)

            def rep_body(_i=None):
                for ci in range(NCHUNK):
                    m0 = ci * MCH
                    if "in" in parts:
                        x_b = xp.tile([128, CIN], bf16, name="x_b")
                        nc.sync.dma_start(x_b[:, :], xs_d[m0:m0 + 128, :])
                    else:
                        x_b = x_s
                    if "drain" in parts:
                        o = op.tile([MCH, C], bf16, name="o")
                    else:
                        o = o_s
                    if "mm" in parts:
                        for blk in range(NBLK):
                            j0 = blk * 512
                            ps = pp.tile([128, 512], f32, name="ps")
                            for dj in range(n_dj):
                                nc.tensor.matmul(
                                    ps[:, :],
                                    w_t[:, dj, :],
                                    x_b[:, j0 + dj:j0 + dj + 512],
                                    start=(dj == 0),
                                    stop=(dj == n_dj - 1),
                                )
                            if "drain" in parts:
                                nc.vector.tensor_scalar_add(
                                    o[:, j0:j0 + 512],
                                    ps[0:MCH, :],
                                    bias_bc[0:MCH, 0:1],
                                )
                    if "out" in parts:
                        nc.scalar.dma_start(
                            ys_d[m0:m0 + MCH, :], o[:, :]
                        )

            if hw_loop and reps > 1:
                tc.For_i_unrolled(0, reps, 1, rep_body, max_unroll=8)
            else:
                for _rep in range(reps):
                    rep_body()

    nc.compile()
    return nc


def _toeplitz_stack(weight: np.ndarray) -> np.ndarray:
    """wT[k, dj, m] = weight[k-m, dj] for 0 <= k-m < KH (m < MCH; cols
    MCH..127 are zero padding so LDWEIGHTS uses the fast-weight-load path)."""
    wT = np.zeros((128, KW, 128), dtype=np.float32)
    for di in range(KH):
        for m in range(MCH):
            wT[m + di, :, m] = weight[di, :]
    return wT


def _prepare_in_maps(x, weight, bias):
    bf16 = _bf16()
    x = np.ascontiguousarray(x, dtype=np.float32)
    weight = np.asarray(weight, dtype=np.float32)
    bias_v = np.asarray(bias, dtype=np.float32).reshape(-1)[:1]

    x_pad = np.zeros((XR_PAD, XC_PAD), dtype=np.float32)
    x_pad[:H, :W] = x
    x_pad = x_pad.astype(bf16)
    wT = _toeplitz_stack(weight).astype(bf16)
    bias_in = bias_v.reshape(1, 1)

    in_maps = []
    for core in range(NCORES):
        c, r = core // HSH, core % HSH
        xs = x_pad[r * B:r * B + BIN, c * C:c * C + CIN]
        in_maps.append(
            {"xs": np.ascontiguousarray(xs), "wT": wT, "bias": bias_in}
        )
    return in_maps


def kernel(x: np.ndarray, weight: np.ndarray, bias: np.ndarray) -> np.ndarray:
    from concourse.bass_utils import run_bass_kernel_spmd

    if "nc" not in _CACHE:
        _CACHE["nc"] = _build_nc()
    nc = _CACHE["nc"]

    in_maps = _prepare_in_maps(x, weight, bias)
    res = run_bass_kernel_spmd(nc, in_maps, core_ids=list(range(NCORES)))

    out = np.empty((HO, WO), dtype=np.float32)
    for core in range(NCORES):
        c, r = core // HSH, core % HSH
        r0, r1 = r * B, min(r * B + B, HO)
        c0, c1 = c * C, min(c * C + C, WO)
        ys = res.results[core]["ys"]
        out[r0:r1, c0:c1] = ys[: r1 - r0, : c1 - c0].astype(np.float32)
    return out
